# revision 1
# baseline (speedup 1.0000x reference)
"""Cross-attention kernel for Trainium2, 8 NeuronCores.

Sharding: core c handles batch b=c//2 and query-half th=c%2 (1024 of 2048
query rows), all 16 heads, full T_E=1024 keys. No cross-core reduction is
needed: each core produces complete [1024, 1024] slices of both outputs.

Per-core pipeline (feature-major "transposed" layouts throughout):
  P0: PE-transpose x-half and enc into xT/encT (f32r, feature-major)
  P1: qT = Wq.T @ xT, kT = Wk.T @ encT (fp16, [co, t/te]);
      v = encT.T @ Wv (+bv via ones-row trick) -> v_aug fp16 with ones col
  P2: per head pair: attT = kT_h.T @ qT_h (K=64, packed via tile_position);
      exp on ACT -> attT_exp fp16; av matmul with v_aug (M=65: 64 y rows +
      softmax-sums row); reciprocal (fp16) -> gpsimd partition-broadcast R;
      y-norm into yT; att_mean accumulators += attT_exp * R (DVE fp16,
      two accumulators to break the serial chain)
  P3: y = yT_aug.T @ Wp_aug (bp via ones-row); att_mean = (acc_a + acc_b)
      transposed back to natural [t, te] on PE, scaled by 1/16.

mask is all-False per the input spec (fill=zeros); if a nonzero mask is ever
passed, a numpy fallback computes the exact reference instead.
"""

import sys

sys.path.insert(0, "/opt/trn_rl_repo")

import numpy as np

import concourse.bass as bass
import concourse.tile as tile
from concourse import bacc, mybir
from concourse.bass_utils import run_bass_kernel_spmd
from concourse.masks import make_identity

F32 = mybir.dt.float32
F32R = mybir.dt.float32r
F16 = mybir.dt.float16

T = 1024   # local query rows per core
TE = 1024  # encoder tokens
C = 1024   # embed dim
H = 16     # heads
DH = 64    # head dim
SCALE = 0.125  # 1/sqrt(64)

_NC = {}


def _build(stages=6, reps=1, vp_bias=True):
    """stages: 1=P0 only, 2=+projections, 3=+qk/exp, 4=+av, 5=+head tails,
    6=full (output proj + att_mean transpose). Partial builds are only for
    timeline-sim bisection."""
    nc = bacc.Bacc("TRN2", target_bir_lowering=False, debug=False, num_devices=8)
    xh = nc.declare_dram_parameter("xh", [T, C], F16, isOutput=False)
    enc = nc.declare_dram_parameter("enc", [TE, C], F16, isOutput=False)
    wq_d = nc.declare_dram_parameter("wq", [C, C], F16, isOutput=False)
    wk_d = nc.declare_dram_parameter("wk", [C, C], F16, isOutput=False)
    wv_d = nc.declare_dram_parameter("wv", [C, C], F16, isOutput=False)
    wp_d = nc.declare_dram_parameter("wp", [C, C], F16, isOutput=False)
    bq_d = nc.declare_dram_parameter("bq", [1, C], F32, isOutput=False)
    bk_d = nc.declare_dram_parameter("bk", [1, C], F32, isOutput=False)
    bv_d = nc.declare_dram_parameter("bv", [1, C], F16, isOutput=False)
    bp_d = nc.declare_dram_parameter("bp", [1, C], F16, isOutput=False)
    y_d = nc.declare_dram_parameter("y", [T, C], F32, isOutput=True)
    am_d = nc.declare_dram_parameter("am", [T, TE], F32, isOutput=True)

    Exp = mybir.ActivationFunctionType.Exp
    Ident = mybir.ActivationFunctionType.Identity

    with tile.TileContext(nc) as tc:
      for _rep in range(reps):
        with tc.tile_pool(name="persist", bufs=1) as persist:
            ident16 = persist.tile([128, 128], F16)
            make_identity(nc, ident16)

            qT = persist.tile([128, 8, T], F16)     # [co%128, co//128, t]
            kT = persist.tile([128, 8, TE], F16)    # [co%128, co//128, te]
            v_aug = persist.tile([128, 8, H, 66], F16)  # [te%128, te//128, h, .]
            yT = persist.tile([128, 8, T], F16)     # [c%128, c//128, t]
            am_a = persist.tile([128, 8, T], F16)   # [te%128, te//128, t]
            ones_t = persist.tile([1, T], F16)      # proj bias row (lhsT)
            bq_sb = persist.tile([128, 8], F32)
            bk_sb = persist.tile([128, 8], F32)
            bv_row = persist.tile([1, C], F16)      # rhs bias row for v
            bp_row = persist.tile([1, C], F16)      # rhs bias row for proj
            ones128 = persist.tile([1, 128], F16)   # lhsT ones row for v bias

            nc.vector.memset(v_aug[:, :, :, 64:66], 0.0)
            nc.vector.memset(v_aug[:, :, :, 64:65], 1.0)
            nc.vector.memset(ones_t, 1.0)
            nc.vector.memset(ones128, 1.0)

            nc.sync.dma_start(out=bq_sb,
                              in_=bq_d[0, :].rearrange("(a p) -> p a", p=128))
            nc.sync.dma_start(out=bk_sb,
                              in_=bk_d[0, :].rearrange("(a p) -> p a", p=128))
            nc.sync.dma_start(out=bv_row, in_=bv_d[:, :])
            nc.sync.dma_start(out=bp_row, in_=bp_d[:, :])

            def _load_weight(pool, w_dram, conv_engine, nbufs=2):
                wbuf = pool.tile([128, 8, C], F16, tag="wbuf", bufs=nbufs)
                nc.sync.dma_start(
                    out=wbuf,
                    in_=w_dram[:, :].rearrange("(a p) c -> p a c", p=128))
                return wbuf

            def _transpose_in(src, dst, p0, ps0):
                for cj in range(8):
                    nc.sync.dma_start_transpose(
                        out=dst[:, cj, :],
                        in_=src[:, cj * 128:(cj + 1) * 128])

            with tc.tile_pool(name="pwp", bufs=1) as pwp:
              wp_buf = _load_weight(pwp, wp_d, "act", nbufs=1)
              with tc.tile_pool(name="psA", bufs=3, space="PSUM") as psA:
                with tc.tile_pool(name="tp", bufs=1) as tp_pool:
                    ps0 = psA
                    encT = tp_pool.tile([128, 8, TE], F16)
                    with tc.tile_pool(name="tpx", bufs=1) as tpx_pool:
                        xT = tpx_pool.tile([128, 8, T], F16)

                        if True:
                            # ------------ P0: transpose x and enc ------------
                            _transpose_in(xh, xT, None, ps0)
                            _transpose_in(enc, encT, None, ps0)

                        # ------------ P1: projections (same scope so
                        # weight loads overlap transposes) ------------
                        if stages >= 2:
                            for (w_dram, lhs_src, dst, b_sb) in (
                                (wq_d, xT, qT, bq_sb),
                                (wk_d, encT, kT, bk_sb),
                            ):
                                wbuf = _load_weight(tpx_pool, w_dram, "dve")
                                for co in range(8):
                                    psq = ps0.tile([128, 1024], F32, tag="ps12", bufs=2)
                                    for t2 in range(2):
                                        for ci in range(8):
                                            nc.tensor.matmul(
                                                psq[:, t2 * 512:(t2 + 1) * 512],
                                                wbuf[:, ci, co * 128:(co + 1) * 128],
                                                lhs_src[:, ci, t2 * 512:(t2 + 1) * 512],
                                                start=(ci == 0),
                                                stop=(ci == 7),
                                            )
                                    nc.scalar.activation(
                                        out=dst[:, co, :], in_=psq, func=Ident,
                                        bias=b_sb[:, co:co + 1], scale=1.0,
                                    )

                            # v natural [te, co] + bv, into v_aug fp16
                            wbuf = _load_weight(tpx_pool, wv_d, "dve")
                            for te in range(8):
                                psq = ps0.tile([128, 1024], F32, tag="ps12", bufs=2)
                                for c2 in range(2):
                                    for ci in range(8):
                                        nc.tensor.matmul(
                                            psq[:, c2 * 512:(c2 + 1) * 512],
                                            encT[:, ci, te * 128:(te + 1) * 128],
                                            wbuf[:, ci, c2 * 512:(c2 + 1) * 512],
                                            start=(ci == 0),
                                            stop=(ci == 7 and not vp_bias),
                                        )
                                    if vp_bias:
                                        nc.tensor.matmul(
                                            psq[:, c2 * 512:(c2 + 1) * 512],
                                            ones128,
                                            bv_row[:, c2 * 512:(c2 + 1) * 512],
                                            start=False,
                                            stop=True,
                                        )
                                nc.vector.tensor_copy(v_aug[:, te, :, 0:64], psq)

                # ---------------- P2: attention ----------------
                if stages >= 3:
                    ps2 = psA
                    with tc.tile_pool(name="p2", bufs=2) as p2:
                        for hpair in range(H // 2):
                            ct = hpair  # co-tile holding heads 2hp, 2hp+1
                            attxs = []
                            for _g in range(2):
                                attx_g = p2.tile([128, 8, T], F16,
                                                 tag="attx", bufs=4)
                                attxs.append(attx_g)
                            # qk for both heads, interleaved so the two K=64
                            # row-groups run concurrently on the PE array
                            for te in range(8):
                                psas = []
                                for _g in range(2):
                                    psa_g = ps2.tile([128, 1024], F32,
                                                     tag="ps12", bufs=2)
                                    psas.append(psa_g)
                                for t2 in range(2):
                                    for g, hp in enumerate((0, 64)):
                                        nc.tensor.matmul(
                                            psas[g][:, t2 * 512:(t2 + 1) * 512],
                                            kT[hp:hp + 64, ct,
                                               te * 128:(te + 1) * 128],
                                            qT[hp:hp + 64, ct,
                                               t2 * 512:(t2 + 1) * 512],
                                            start=True, stop=True,
                                            tile_position=(hp, 0),
                                        )
                                for g in range(2):
                                    nc.scalar.activation(
                                        out=attxs[g][:, te, :], in_=psas[g],
                                        func=Exp, scale=SCALE,
                                    )
                            if stages < 4:
                                continue
                            psys, Rs = [], []
                            for g, hp in enumerate((0, 64)):
                                h = 2 * hpair + g
                                attx = attxs[g]
                                psy = ps2.tile([65, 1024], F32, tag="psy", bufs=2)
                                psys.append(psy)
                                for te in range(8):
                                    for t2 in range(2):
                                        nc.tensor.matmul(
                                            psy[:, t2 * 512:(t2 + 1) * 512],
                                            v_aug[:, te, h, 0:65],
                                            attx[:, te, t2 * 512:(t2 + 1) * 512],
                                            start=(te == 0), stop=(te == 7),
                                        )
                                if stages < 5:
                                    continue
                                r16 = p2.tile([1, T], F16, tag="r16", bufs=2)
                                with nc.allow_low_precision("softmax recip fp16"):
                                    nc.vector.reciprocal(r16, psy[64:65, :])
                                R = p2.tile([128, T], F16, tag="R", bufs=2)
                                nc.gpsimd.partition_broadcast(R, r16)
                                Rs.append(R)
                            if stages < 5:
                                continue
                            for g, hp in enumerate((0, 64)):
                                attx, psy, R = attxs[g], psys[g], Rs[g]
                                # normalized y head -> yT
                                nc.vector.tensor_mul(
                                    yT[hp:hp + 64, ct, :], psy[0:64, :], R[0:64, :]
                                )
                                # att_mean accumulators += attx * R
                                R_b = bass.AP(
                                    tensor=R.tensor, offset=R.offset,
                                    ap=[R.ap[0], [0, 8], R.ap[1]],
                                )
                                if hpair == 0 and g == 0:
                                    # very first head writes the accumulator
                                    nc.vector.tensor_mul(am_a, attx, R_b)
                                else:
                                    tmp = p2.tile([128, 8, T], F16, tag="tmp", bufs=2)
                                    nc.vector.tensor_mul(tmp, attx, R_b)
                                    nc.vector.tensor_add(am_a, am_a, tmp)

              # ------------ P3: output proj + att_mean transpose ------
              if stages >= 6:
                  with tc.tile_pool(name="p3", bufs=2) as p3, \
                       tc.tile_pool(name="ps3", bufs=2, space="PSUM") as ps3:
                      am_s = am_a
                      for tt in range(8):
                          pso = ps3.tile([128, 1024], F32, tag="pso")
                          for c2 in range(2):
                              for ci in range(8):
                                  nc.tensor.matmul(
                                      pso[:, c2 * 512:(c2 + 1) * 512],
                                      yT[:, ci, tt * 128:(tt + 1) * 128],
                                      wp_buf[:, ci, c2 * 512:(c2 + 1) * 512],
                                      start=(ci == 0),
                                      stop=(ci == 7 and not vp_bias),
                                  )
                              if vp_bias:
                                  nc.tensor.matmul(
                                      pso[:, c2 * 512:(c2 + 1) * 512],
                                      ones_t[:, tt * 128:(tt + 1) * 128],
                                      bp_row[:, c2 * 512:(c2 + 1) * 512],
                                      start=False, stop=True,
                                  )
                          yo = p3.tile([128, 1024], F32, tag="yo")
                          nc.scalar.copy(yo, pso)
                          nc.sync.dma_start(
                              out=y_d[tt * 128:(tt + 1) * 128, :], in_=yo
                          )
                          # interleaved att_mean transpose for the same tt
                          psm = ps3.tile([128, 1024], F16, tag="psm")
                          for te in range(8):
                              nc.tensor.transpose(
                                  psm[:, te * 128:(te + 1) * 128],
                                  am_s[:, te, tt * 128:(tt + 1) * 128],
                                  ident16,
                              )
                          mo = p3.tile([128, 1024], F32, tag="mo")
                          nc.scalar.mul(mo, psm, 1.0 / H)
                          nc.sync.dma_start(
                              out=am_d[tt * 128:(tt + 1) * 128, :], in_=mo
                          )

    nc.finalize()
    return nc


def _get_nc(vp_bias=False):
    if vp_bias not in _NC:
        _NC[vp_bias] = _build(vp_bias=vp_bias)
    return _NC[vp_bias]


def _numpy_fallback(x, enc, mask, wq, bq, wk, bk, wv, bv, wp, bp):
    B, Tt, Cc = x.shape
    q = (x @ wq + bq).reshape(B, Tt, H, DH)
    k = (enc @ wk + bk).reshape(B, enc.shape[1], H, DH)
    v = (enc @ wv + bv).reshape(B, enc.shape[1], H, DH)
    att = np.einsum("bqhd,bkhd->bhqk", q, k).astype(np.float32) * SCALE
    att = np.where(mask[:, None, :, :], -np.inf, att)
    att = att - att.max(axis=-1, keepdims=True)
    att = np.exp(att)
    att = att / att.sum(axis=-1, keepdims=True)
    y = np.einsum("bhqk,bkhd->bqhd", att, v).reshape(B, Tt, Cc)
    am = att.mean(axis=1)
    y = y @ wp + bp
    return y.astype(np.float32), am.astype(np.float32)


def _run(inputs, trace=False):
    x = np.asarray(inputs["x"], dtype=np.float32)
    enc = np.asarray(inputs["encoder_output"], dtype=np.float32)
    mask = np.asarray(inputs["mask"])
    wq = np.asarray(inputs["Wq"], dtype=np.float32)
    wk = np.asarray(inputs["Wk"], dtype=np.float32)
    wv = np.asarray(inputs["Wv"], dtype=np.float32)
    wp = np.asarray(inputs["Wp"], dtype=np.float32)
    bq = np.asarray(inputs["bq"], dtype=np.float32).reshape(1, C)
    bk = np.asarray(inputs["bk"], dtype=np.float32).reshape(1, C)
    bv = np.asarray(inputs["bv"], dtype=np.float32).reshape(1, C)
    bp = np.asarray(inputs["bp"], dtype=np.float32).reshape(1, C)
    x16 = np.ascontiguousarray(x.astype(np.float16))
    enc16 = np.ascontiguousarray(enc.astype(np.float16))
    w16 = {n: np.ascontiguousarray(w.astype(np.float16))
           for n, w in (("wq", wq), ("wk", wk), ("wv", wv), ("wp", wp))}

    if mask.any():
        return _numpy_fallback(x, enc, mask,
                               wq, bq[0], wk, bk[0], wv, bv[0],
                               wp, bp[0]), None

    nc = _get_nc(vp_bias=bool(bv.any() or bp.any()))
    in_maps = []
    for c in range(8):
        b, th = divmod(c, 2)
        in_maps.append({
            "xh": x16[b, th * T:(th + 1) * T],
            "enc": enc16[b],
            "wq": w16["wq"], "wk": w16["wk"],
            "wv": w16["wv"], "wp": w16["wp"],
            "bq": bq, "bk": bk,
            "bv": bv.astype(np.float16), "bp": bp.astype(np.float16),
        })
    res = run_bass_kernel_spmd(nc, in_maps, core_ids=list(range(8)),
                               trace=trace)
    B = x.shape[0]
    y = np.empty((B, 2 * T, C), np.float32)
    am = np.empty((B, 2 * T, TE), np.float32)
    for c in range(8):
        b, th = divmod(c, 2)
        y[b, th * T:(th + 1) * T] = res.results[c]["y"]
        am[b, th * T:(th + 1) * T] = res.results[c]["am"]
    return (y, am), res


def kernel(**inputs):
    out, _ = _run(inputs, trace=False)
    return out

